# revision 13
# baseline (speedup 1.0000x reference)
"""Trainium2 SPMD kernel for nn_CombinedGeneModel.

Math (per batch b, tech t, gene g; R = T*G independent tiny MLPs):
    h   = relu(x * w1[r,e] + b1[r,e])          e = 0..3
    s   = relu(sum_e h*w2[r,e] + b2[r])
    out = relu(sum_t s[b,t,g]*wg[g,t] + bg[g])

With b1 == 0 (guaranteed by setup_inputs) the E=4 hinge sum folds exactly:
    sum_e w2_e*relu(w1_e*x) = c*relu(x) + d*x
      c = sum_e w2_e*|w1_e|,  d = sum_e w2_e*min(w1_e, 0)
so per row:  s = relu(c*relu(x) + d*x + b2)   -> 3 cheap vector ops.

Layout: genes on SBUF partitions (all weights become per-partition scalars
for tensor_scalar / scalar_tensor_tensor), batch on the free axis.  Host
transposes x to [G, T, B] fp16 so every DMA is contiguous; genes are
sharded across the 8 NeuronCores.
"""

import os
import numpy as np

N_GENES = 20000
N_TECH = 2
BATCH = 1024
N_CORES = 8
P = 128
G_PAD = 20480            # next multiple of 8*128 above 20000
GS = G_PAD // N_CORES    # 2560 genes per core
NTILES = GS // P         # 20 tiles of 128 genes
FD = BATCH               # free dim per (tile, tech)
STORE_CHUNK = 10         # tiles per output store DMA (2 stores -> 2 SWDGE lanes, keeps the kernel-tail Drain under the sync-wait limit)

LAST_EXEC_NS = None
LAST_RESULTS = None

_nc_cache = {}


def _build_nc(has_b2: bool):
    import concourse.bass as bass
    import concourse.mybir as mybir
    from concourse.tile import TileContext

    Op = mybir.AluOpType
    Act = mybir.ActivationFunctionType
    f16 = mybir.dt.float16
    f32 = mybir.dt.float32

    ncol = 9 if has_b2 else 7  # per-tile scalar columns

    nc = bass.Bass()
    x_d = nc.declare_dram_parameter("x", [NTILES, P, 2 * FD], f16, isOutput=False)
    w_d = nc.declare_dram_parameter("w", [P, NTILES * ncol], f32, isOutput=False)
    o_d = nc.declare_dram_parameter("out", [NTILES, P, FD], f16, isOutput=True)

    with TileContext(nc) as tc:
        with (
            tc.tile_pool(name="wp", bufs=1) as wpool,
            tc.tile_pool(name="xp", bufs=NTILES) as xpool,
            tc.tile_pool(name="op", bufs=1) as opool,
            tc.tile_pool(name="tp", bufs=2) as tpool,
            tc.tile_pool(name="sp1", bufs=NTILES) as s1pool,
        ):
            # one big contiguous output staging buffer; stores go out in
            # STORE_CHUNK-tile chunks so each lands on a fresh SWDGE lane
            # (every DMA may carry at most one sync wait in this walrus).
            obuf = opool.tile([P, NTILES * FD], f16)
            w = wpool.tile([P, NTILES * ncol], f32)
            nc.sync.dma_start(w[:], w_d[:])
            # "touch" w once per compute engine so the w-DMA semaphore wait
            # is absorbed here; later ops then carry at most one wait each
            # (walrus rejects TensorScalarPtr with >1 sync wait).
            wt_v = wpool.tile([P, 1], f32)
            nc.vector.tensor_copy(wt_v[:], w[:, 0:1])
            wt_a = wpool.tile([P, 1], f32)
            nc.scalar.copy(wt_a[:], w[:, 0:1])

            for j in range(NTILES):
                col = j * ncol
                c0 = w[:, col + 0 : col + 1]
                d0 = w[:, col + 1 : col + 2]
                c1 = w[:, col + 2 : col + 3]
                d1 = w[:, col + 3 : col + 4]
                wg0 = w[:, col + 4 : col + 5]
                wg1 = w[:, col + 5 : col + 6]
                bg = w[:, col + 6 : col + 7]

                xt = xpool.tile([P, 2 * FD], f16, tag="x")
                nc.sync.dma_start(xt[:], x_d[j])
                x0 = xt[:, 0:FD]
                x1 = xt[:, FD : 2 * FD]

                # tech 0: s0 = wg0 * relu(c0*relu(x0) + d0*x0 [+ b20])
                p0 = tpool.tile([P, FD], f16, tag="p0")
                nc.vector.tensor_scalar(p0[:], x0, 0.0, c0, Op.max, Op.mult)
                u0 = tpool.tile([P, FD], f16, tag="u0")
                if has_b2:
                    b20 = w[:, col + 7 : col + 8]
                    t0 = tpool.tile([P, FD], f16, tag="t0")
                    nc.vector.tensor_scalar(t0[:], x0, d0, b20, Op.mult, Op.add)
                    nc.vector.tensor_tensor(u0[:], t0[:], p0[:], Op.add)
                else:
                    nc.vector.scalar_tensor_tensor(u0[:], x0, d0, p0[:], Op.mult, Op.add)
                s0 = tpool.tile([P, FD], f16, tag="s0")
                nc.vector.tensor_scalar(s0[:], u0[:], 0.0, wg0, Op.max, Op.mult)

                # tech 1: s1r = relu(c1*relu(x1) + d1*x1 [+ b21]) on ACT,
                # wg1 folded into the combine below.
                p1 = tpool.tile([P, FD], f16, tag="p1")
                nc.vector.tensor_scalar(p1[:], x1, 0.0, c1, Op.max, Op.mult)
                u1 = tpool.tile([P, FD], f16, tag="u1")
                if has_b2:
                    b21 = w[:, col + 8 : col + 9]
                    t1 = tpool.tile([P, FD], f16, tag="t1")
                    nc.vector.tensor_scalar(t1[:], x1, d1, b21, Op.mult, Op.add)
                    nc.vector.tensor_tensor(u1[:], t1[:], p1[:], Op.add)
                else:
                    nc.vector.scalar_tensor_tensor(u1[:], x1, d1, p1[:], Op.mult, Op.add)
                s1 = s1pool.tile([P, FD], f16, tag="s1")
                nc.scalar.activation(s1[:], u1[:], Act.Relu)

                # combine: out = relu(wg1*s1 + s0 + bg)
                o = tpool.tile([P, FD], f16, tag="o")
                if j == NTILES - 1:
                    # scrap read of s0 bumps DVE's observed clock so the stt
                    # below needs only the ACT wait (1-wait-per-inst limit)
                    scrap = tpool.tile([P, 1], f16, tag="scrap")
                    nc.vector.tensor_copy(scrap[:], s0[:, 0:1])
                nc.vector.scalar_tensor_tensor(o[:], s1[:], wg1, s0[:], Op.mult, Op.add)
                ot = obuf[:, j * FD : (j + 1) * FD]
                nc.scalar.activation(ot, o[:], Act.Relu, bias=bg)
                if (j + 1) % STORE_CHUNK == 0:
                    k0 = j + 1 - STORE_CHUNK
                    src = obuf[:, k0 * FD : (j + 1) * FD].rearrange(
                        "p (t b) -> p t b", t=STORE_CHUNK
                    )
                    dst = o_d[k0 : j + 1].rearrange("t p b -> p t b")
                    nc.gpsimd.dma_start(dst, src)

    _split_multi_waits(nc, mybir)
    return nc


def _split_multi_waits(nc, mybir):
    """walrus (gen3 codegen here) accepts at most one sync wait per
    instruction.  Tile's epilogue Drain aggregates every outstanding
    semaphore into one instruction; hoist all but one wait onto
    same-engine NoOps appended to the preceding basic block."""
    blocks = list(nc.main_func.blocks)
    nop_idx = 0
    for bi, bb in enumerate(blocks):
        for ins in bb.instructions:
            si = getattr(ins, "sync_info", None)
            if si is None:
                continue
            waits = list(si.on_wait or [])
            if len(waits) <= 1:
                continue
            assert bi > 0, "multi-wait instruction in first block"
            # the instruction must be the first of its engine in this block
            # so that appending NoOps to the previous block keeps ordering
            for other in bb.instructions:
                if other.name == ins.name:
                    break
                assert other.engine != ins.engine
            prev_bb = blocks[bi - 1]
            for w in waits[:-1]:
                nop = mybir.InstNoOp(name=f"ant-waitsplit-{nop_idx}")
                nop_idx += 1
                nop.engine = ins.engine
                nop.sync_info = mybir.SyncInfo(on_wait=[w], on_update=[])
                prev_bb.add_instruction(nop)
            ins.sync_info = mybir.SyncInfo(
                on_wait=[waits[-1]], on_update=list(si.on_update or [])
            )


def _numpy_fallback(x, w1, b1, w2, b2, wg, bgv):
    B = x.shape[0]
    R = N_GENES * N_TECH
    xr = x.reshape(B, R).T.astype(np.float32)
    h = np.maximum(xr[:, :, None] * w1[:, None, :] + b1[:, None, :], 0.0)
    s = np.maximum(np.einsum("rbe,re->rb", h, w2) + b2[:, None], 0.0)
    s = s.T.reshape(B, N_TECH, N_GENES)
    out = np.maximum(np.einsum("btg,gt->bg", s, wg) + bgv, 0.0)
    return out.astype(np.float32)


def kernel(x, weights1, bias1, weights2, bias2, weights_g, bias_g):
    global LAST_EXEC_NS, LAST_RESULTS
    x = np.asarray(x, dtype=np.float32)
    w1 = np.asarray(weights1, dtype=np.float32)
    b1 = np.asarray(bias1, dtype=np.float32)
    w2 = np.asarray(weights2, dtype=np.float32)
    b2 = np.asarray(bias2, dtype=np.float32)
    wg = np.asarray(weights_g, dtype=np.float32)
    bgv = np.asarray(bias_g, dtype=np.float32)

    if np.any(b1 != 0.0):
        # hinge-folding below needs b1 == 0; exact general fallback
        return _numpy_fallback(x, w1, b1, w2, b2, wg, bgv)

    # fold the E=4 expand/shrink into two per-row coefficients
    c = (w2 * np.abs(w1)).sum(axis=1)          # [R]
    d = (w2 * np.minimum(w1, 0.0)).sum(axis=1)  # [R]
    has_b2 = bool(np.any(b2 != 0.0))

    # per-gene scalar table, padded to G_PAD
    ncol = 9 if has_b2 else 7
    wtab = np.zeros((G_PAD, ncol), dtype=np.float32)
    G = N_GENES
    wtab[:G, 0] = c[:G]
    wtab[:G, 1] = d[:G]
    wtab[:G, 2] = c[G:]
    wtab[:G, 3] = d[G:]
    wtab[:G, 4] = wg[:, 0]
    wtab[:G, 5] = wg[:, 1]
    wtab[:G, 6] = bgv
    if has_b2:
        wtab[:G, 7] = b2[:G]
        wtab[:G, 8] = b2[G:]

    # x -> [G_PAD, T, B] fp16, contiguous per gene row
    xt = np.zeros((G_PAD, N_TECH, BATCH), dtype=np.float16)
    xt[:G] = x.transpose(2, 1, 0)

    in_maps = []
    for i in range(N_CORES):
        g0 = i * GS
        xi = np.ascontiguousarray(
            xt[g0 : g0 + GS].reshape(NTILES, P, 2 * FD)
        )
        wi = np.ascontiguousarray(
            wtab[g0 : g0 + GS].reshape(NTILES, P, ncol).transpose(1, 0, 2)
            .reshape(P, NTILES * ncol)
        )
        in_maps.append({"x": xi, "w": wi})

    key = has_b2
    if key not in _nc_cache:
        _nc_cache[key] = _build_nc(has_b2)
    nc = _nc_cache[key]

    from concourse.bass_utils import run_bass_kernel_spmd

    trace = bool(int(os.environ.get("KERNEL_TRACE", "0")))
    res = run_bass_kernel_spmd(nc, in_maps, core_ids=list(range(N_CORES)),
                               trace=trace)
    LAST_EXEC_NS = res.exec_time_ns
    LAST_RESULTS = res

    parts = [res.results[i]["out"].reshape(GS, BATCH) for i in range(N_CORES)]
    full = np.concatenate(parts, axis=0)[:G]          # [G, B] fp16
    return np.ascontiguousarray(full.T).astype(np.float32)
